# revision 43
# baseline (speedup 1.0000x reference)
"""Trainium2 Bass kernel for the LVIS-style masked sigmoid-BCE loss.

Computes, for cls_logits [16384, 1231] (+ label / mask / sel inputs):
    loss = sum(wm * (softplus(x) - x * onehot(labels))) / n_i
with the weight mask wm built from a score threshold, per-class group
masks, background sampling masks, and label columns.

Strategy (data-parallel over the 8 NeuronCores, 2048 rows each):
    wm = max(c, s, eq):  c = fg * (x >= thr), s = u[t_i, j] (outer
    product of 3 row-sel bits with 3 column masks, OR-combined),
    eq = onehot(label).
    sum(wm * A) with A = softplus(x) decomposes into
        sum over (t, j) of [ u * P1 + (1 - u) * P2 ]  +  per-row terms,
    where P1 = R^T A and P2 = (fg*R)^T (c' * A) are 8 x n_c matmul
    accumulations over all row tiles (R = onehot of the 8 sel-bit row
    types).  The per-row terms only need g_i = x[i, label_i], gathered
    with indirect DMA.  sum(wm * eq * x) = sum(g).
Per 256-row supertile the device does: 1 bf16 DMA load, ACT Exp +
ACT Ln(1+e) (softplus, batched into two table phases so the Exp/Ln
activation tables load twice per kernel), one DVE bf16 threshold
compare + one DVE bf16 multiply, and 12 PE matmuls accumulating into
PSUM.  Measured ~63 us on hardware per core (HBM roofline ~28 us for
f32; x is streamed bf16, ACT softplus at 2 passes/element is the
binding engine at ~40 us busy).
"""

import math
from contextlib import ExitStack

import numpy as np
import ml_dtypes

import concourse.bass as bass
import concourse.tile as tile
from concourse import bacc, mybir
from concourse.bass_utils import run_bass_kernel_spmd

N_I, N_C = 16384, 1231
N_CORES = 8
N_LOC = N_I // N_CORES          # 2048 rows per core
P = 128
K_TILES = N_LOC // P            # 16 row tiles per core
SUPER = 2                       # row tiles per supertile (ACT/DVE instr batching)
N_SUPER = K_TILES // SUPER
THR = float(math.log(0.7 / 0.3))  # sigmoid(x) >= 0.7  <=>  x >= THR
TAU = float(math.log(1.0 + 0.7 / 0.3))  # softplus(THR): x >= THR <=> softplus(x) >= TAU
N_CHUNKS = [(0, 512), (512, 1024), (1024, N_C)]  # PSUM-bank-sized matmul slices

F32 = mybir.dt.float32
BF16 = mybir.dt.bfloat16
I32 = mybir.dt.int32


def _build_nc():
    nc = bacc.Bacc(None, target_bir_lowering=False)
    # x streamed as bf16: halves HBM traffic; softplus/threshold precision
    # impact is ~1e-4 relative (unbiased rounding), far inside tolerance
    x = nc.dram_tensor("x", [N_LOC, N_C], BF16, kind="ExternalInput")
    r_d = nc.dram_tensor("r", [P, K_TILES, 8], BF16, kind="ExternalInput")
    rp_d = nc.dram_tensor("rp", [P, K_TILES, 8], BF16, kind="ExternalInput")
    u_d = nc.dram_tensor("u", [8, N_C], F32, kind="ExternalInput")
    uc_d = nc.dram_tensor("uc", [8, N_C], F32, kind="ExternalInput")
    a_d = nc.dram_tensor("wa", [P, K_TILES], F32, kind="ExternalInput")
    b_d = nc.dram_tensor("wb", [P, K_TILES], F32, kind="ExternalInput")
    goff_d = nc.dram_tensor("goff", [P, K_TILES], I32, kind="ExternalInput")
    out_d = nc.dram_tensor("out", [1, 1], F32, kind="ExternalOutput")

    xv = x.rearrange("(k p) c -> p k c", p=P)  # [128, K_TILES, N_C]
    x_flat = x.rearrange("r (c one) -> (r c) one", one=1)
    # supertile row-tile counts: small head tiles so ACT starts as soon as
    # the first 630KB lands instead of waiting for a full 1.26MB supertile;
    # small tail tiles so DVE/PE can chase the last Ln sooner
    SIZES = [SUPER] * 7 + [1, 1]
    assert sum(SIZES) == K_TILES
    STARTS = [sum(SIZES[:i]) for i in range(len(SIZES))]
    N_ST = len(SIZES)
    # one ACT table phase pair (all Exp then all Ln) -> 2 table loads;
    # bf16 x makes DMA fast enough that the E-phase is not starved
    PHASES = [list(range(0, N_ST))]

    with tile.TileContext(nc) as tc, ExitStack() as ctx:
        const = ctx.enter_context(tc.tile_pool(name="const", bufs=1))
        xpool = ctx.enter_context(tc.tile_pool(name="x", bufs=1))
        epool = ctx.enter_context(tc.tile_pool(name="e", bufs=1))
        apool = ctx.enter_context(tc.tile_pool(name="a", bufs=1))
        cpool = ctx.enter_context(tc.tile_pool(name="c", bufs=1))
        mpool = ctx.enter_context(tc.tile_pool(name="m", bufs=1))
        psum = ctx.enter_context(tc.tile_pool(name="psum", bufs=1, space="PSUM"))
        fin = ctx.enter_context(tc.tile_pool(name="fin", bufs=1))

        # first x supertile DMA goes out before any constant loads
        xs_tiles = [None] * N_ST

        def load_xs(s, split=False):
            k0, sz = STARTS[s], SIZES[s]
            xs_tiles[s] = xpool.tile([P, sz, N_C], BF16, tag="xs",
                                     name=f"xs{s}", bufs=4)
            if split and sz == 2:
                # halves on both DGE queues so the first tile lands sooner
                nc.sync.dma_start(xs_tiles[s][:, 0, :], xv[:, k0, :])
                nc.gpsimd.dma_start(xs_tiles[s][:, 1, :], xv[:, k0 + 1, :])
            else:
                nc.sync.dma_start(xs_tiles[s][:], xv[:, k0 : k0 + sz, :])

        load_xs(0, split=True)
        load_xs(1)

        # constants, issued from the otherwise-idle gpsimd queue so the
        # sync sequencer's ~0.6us/issue budget all goes to x tiles
        r_sb = const.tile([P, K_TILES, 8], BF16)
        nc.gpsimd.dma_start(r_sb[:], r_d[:])
        rp_sb = const.tile([P, K_TILES, 8], BF16)
        nc.gpsimd.dma_start(rp_sb[:], rp_d[:])
        goff_sb = const.tile([P, K_TILES], I32)
        nc.gpsimd.dma_start(goff_sb[:], goff_d[:])
        u_sb = const.tile([8, N_C], F32)
        nc.gpsimd.dma_start(u_sb[:], u_d[:])
        uc_sb = const.tile([8, N_C], F32)
        nc.gpsimd.dma_start(uc_sb[:], uc_d[:])
        a_sb = const.tile([P, K_TILES], F32)
        nc.gpsimd.dma_start(a_sb[:], a_d[:])
        b_sb = const.tile([P, K_TILES], F32)
        nc.gpsimd.dma_start(b_sb[:], b_d[:])
        ones = const.tile([P, 1], F32)
        nc.vector.memset(ones[:], 1.0)

        # per-row gathered logits g[p, k] = x[row, label[row]] — one
        # indirect DMA with all 2048 offsets (per-instruction overhead on
        # the gpsimd descriptor generator dominates split gathers)
        g_sb = const.tile([P, K_TILES], BF16)
        nc.gpsimd.indirect_dma_start(
            out=g_sb[:, :],
            out_offset=None,
            in_=x_flat,
            in_offset=bass.IndirectOffsetOnAxis(ap=goff_sb[:, :], axis=0),
        )

        p1 = psum.tile([8, N_C], F32, space="PSUM")
        p2 = psum.tile([8, N_C], F32, space="PSUM")

        eg = fin.tile([P, K_TILES], F32)
        spg = fin.tile([P, K_TILES], F32)

        # ACT instructions batched per phase (Exp xN then Ln xN) so the
        # activation-table swap happens 4x per kernel, not 20x.  The Tile
        # scheduler is table-load-oblivious, so the grouping is pinned
        # with explicit ordering deps between consecutive ACT instrs.
        act_order = []
        dve_order = []
        e_tiles = [None] * N_ST
        a_tiles = [None] * N_ST
        for pi, phase in enumerate(PHASES):
            for s in phase:
                if xs_tiles[s] is None:
                    load_xs(s)
                sz = SIZES[s]
                e_tiles[s] = epool.tile([P, sz, N_C], BF16, tag="e",
                                        name=f"et{s}", bufs=10)
                act_order.append(nc.scalar.activation(
                    e_tiles[s][:], xs_tiles[s][:], mybir.ActivationFunctionType.Exp
                ))
            last = pi == len(PHASES) - 1
            if last:
                # fold the tiny gathered-g softplus into the last phase's tables
                act_order.append(nc.scalar.activation(
                    eg[:], g_sb[:], mybir.ActivationFunctionType.Exp))
            for s in phase:
                sz = SIZES[s]
                a_tiles[s] = apool.tile([P, sz, N_C], BF16, tag="a",
                                        name=f"at{s}", bufs=4)
                act_order.append(nc.scalar.activation(
                    a_tiles[s][:], e_tiles[s][:],
                    mybir.ActivationFunctionType.Ln, bias=1.0,
                ))
            if last:
                act_order.append(nc.scalar.activation(
                    spg[:], eg[:], mybir.ActivationFunctionType.Ln, bias=1.0
                ))
            for s in phase:
                sz = SIZES[s]
                a_t = a_tiles[s]
                c_t = cpool.tile([P, sz, N_C], BF16, tag="c",
                                 name=f"ct{s}", bufs=3)
                dve_order.append(nc.vector.tensor_scalar(
                    c_t[:], a_t[:], TAU, None, mybir.AluOpType.is_ge
                ))
                m_t = mpool.tile([P, sz, N_C], BF16, tag="m",
                                 name=f"mt{s}", bufs=3)
                dve_order.append(nc.vector.tensor_tensor(
                    m_t[:], c_t[:], a_t[:], mybir.AluOpType.mult))
                # all P1 matmuls before P2's: the P2 chunks wait on m_t and
                # would stall the PE stream ahead of the ready P1 work
                for j in range(sz):
                    k = STARTS[s] + j
                    for n0, n1 in N_CHUNKS:
                        nc.tensor.matmul(
                            p1[:, n0:n1], r_sb[:, k, :], a_t[:, j, n0:n1],
                            start=(k == 0), stop=(k == K_TILES - 1),
                        )
                for j in range(sz):
                    k = STARTS[s] + j
                    for n0, n1 in N_CHUNKS:
                        nc.tensor.matmul(
                            p2[:, n0:n1], rp_sb[:, k, :], m_t[:, j, n0:n1],
                            start=(k == 0), stop=(k == K_TILES - 1),
                        )

        # pin the ACT stream order so table-load batching survives scheduling
        for prev, nxt in zip(act_order, act_order[1:]):
            tile.add_dep_helper(nxt.ins, prev.ins, sync=False,
                                reason="ACT table-load grouping")

        # epilogue: sum(u * P1 + (1 - u) * P2); bf16 intermediates let the
        # add run in the DVE 2x mode and shave the serial tail
        t1 = fin.tile([8, N_C], BF16)
        dve_order.append(nc.vector.tensor_tensor(
            t1[:], p1[:], u_sb[:], mybir.AluOpType.mult))
        t2 = fin.tile([8, N_C], BF16)
        dve_order.append(nc.vector.tensor_tensor(
            t2[:], p2[:], uc_sb[:], mybir.AluOpType.mult))
        t3 = fin.tile([8, N_C], BF16)
        dve_order.append(nc.vector.tensor_tensor(
            t3[:], t1[:], t2[:], mybir.AluOpType.add))
        r8 = fin.tile([8, 1], F32)
        dve_order.append(nc.vector.reduce_sum(
            r8[:], t3[:], axis=mybir.AxisListType.X))

        # per-row terms: (wa + wb*[g<thr]) * softplus(g) - g
        g32 = fin.tile([P, K_TILES], F32)
        dve_order.append(nc.vector.tensor_copy(g32[:], g_sb[:]))
        mlt = fin.tile([P, K_TILES], F32)
        dve_order.append(nc.vector.tensor_scalar(
            mlt[:], g32[:], THR, None, mybir.AluOpType.is_lt))
        w1 = fin.tile([P, K_TILES], F32)
        dve_order.append(nc.vector.tensor_tensor(
            w1[:], mlt[:], b_sb[:], mybir.AluOpType.mult))
        w2 = fin.tile([P, K_TILES], F32)
        dve_order.append(nc.vector.tensor_tensor(
            w2[:], w1[:], a_sb[:], mybir.AluOpType.add))
        t4 = fin.tile([P, K_TILES], F32)
        dve_order.append(nc.vector.tensor_tensor(
            t4[:], w2[:], spg[:], mybir.AluOpType.mult))
        t5 = fin.tile([P, K_TILES], F32)
        dve_order.append(nc.vector.tensor_tensor(
            t5[:], t4[:], g32[:], mybir.AluOpType.subtract))
        rr = fin.tile([P, 1], F32)
        dve_order.append(nc.vector.reduce_sum(
            rr[:], t5[:], axis=mybir.AxisListType.X))

        # total = sum(r8) + sum(rr), via ones^T matmuls into one PSUM scalar
        s_ps = psum.tile([1, 1], F32, space="PSUM")
        nc.tensor.matmul(s_ps[:], ones[:8, :], r8[:], start=True, stop=False,
                         skip_group_check=True)
        nc.tensor.matmul(s_ps[:], ones[:], rr[:], start=False, stop=True,
                         skip_group_check=True)
        out_sb = fin.tile([1, 1], F32)
        nc.vector.tensor_copy(out_sb[:], s_ps[:])
        nc.sync.dma_start(out_d[:], out_sb[:])

    nc.finalize()
    return nc


_NC_CACHE = None


def _get_nc():
    global _NC_CACHE
    if _NC_CACHE is None:
        _NC_CACHE = _build_nc()
    return _NC_CACHE


def _prep_in_maps(cls_logits, labels, rare_mask, common_mask, freq_mask,
                  rare_sel, common_sel, freq_sel):
    x = np.ascontiguousarray(
        np.asarray(cls_logits, dtype=np.float32).astype(ml_dtypes.bfloat16))
    lab = np.asarray(labels).astype(np.int64)
    rm = np.asarray(rare_mask).astype(np.float32)
    cm = np.asarray(common_mask).astype(np.float32)
    fm = np.asarray(freq_mask).astype(np.float32)
    rs = np.asarray(rare_sel).astype(np.int64)
    cs = np.asarray(common_sel).astype(np.int64)
    fs = np.asarray(freq_sel).astype(np.int64)

    t = rs + 2 * cs + 4 * fs                      # row type in [0, 8)
    fg = (lab != 0).astype(np.float32)
    R = np.zeros((N_I, 8), np.float32)
    R[np.arange(N_I), t] = 1.0
    Rp = R * fg[:, None]

    u8 = np.zeros((8, N_C), np.float32)
    for tt in range(8):
        m = np.zeros(N_C, np.float32)
        if tt & 1:
            m = np.maximum(m, rm)
        if tt & 2:
            m = np.maximum(m, cm)
        if tt & 4:
            m = np.maximum(m, fm)
        u8[tt] = m

    h = u8[t, lab]                                # s value at the label column
    wa = (1.0 - h) * (1.0 - fg)
    wb = (1.0 - h) * fg

    loc = np.arange(N_LOC, dtype=np.int64)

    def fold(v):  # [N_LOC] -> [P, K_TILES] (partition-major)
        return np.ascontiguousarray(v.reshape(K_TILES, P).T)

    in_maps = []
    for c in range(N_CORES):
        rows = slice(c * N_LOC, (c + 1) * N_LOC)
        goff = loc * N_C + lab[rows]
        in_maps.append({
            "x": x[rows],
            "r": np.ascontiguousarray(
                R[rows].reshape(K_TILES, P, 8).transpose(1, 0, 2)
            ).astype(ml_dtypes.bfloat16),
            "rp": np.ascontiguousarray(
                Rp[rows].reshape(K_TILES, P, 8).transpose(1, 0, 2)
            ).astype(ml_dtypes.bfloat16),
            "u": u8,
            "uc": np.ascontiguousarray(1.0 - u8),
            "wa": fold(wa[rows].astype(np.float32)),
            "wb": fold(wb[rows].astype(np.float32)),
            "goff": fold(goff).astype(np.int32),
        })
    return in_maps


def kernel(cls_logits, labels, rare_mask, common_mask, freq_mask,
           rare_sel, common_sel, freq_sel, _trace=False):
    in_maps = _prep_in_maps(cls_logits, labels, rare_mask, common_mask,
                            freq_mask, rare_sel, common_sel, freq_sel)
    nc = _get_nc()
    res = run_bass_kernel_spmd(nc, in_maps, core_ids=list(range(N_CORES)),
                               trace=_trace)
    total = np.float32(0.0)
    for c in range(N_CORES):
        total += res.results[c]["out"].reshape(())
    out = np.asarray(total / np.float32(N_I), dtype=np.float32)
    if _trace:
        return out, res
    return out


# revision 44
# speedup vs baseline: 1.0630x; 1.0630x over previous
"""Trainium2 Bass kernel for the LVIS-style masked sigmoid-BCE loss.

Computes, for cls_logits [16384, 1231] (+ label / mask / sel inputs):
    loss = sum(wm * (softplus(x) - x * onehot(labels))) / n_i
with the weight mask wm built from a score threshold, per-class group
masks, background sampling masks, and label columns.

Strategy (data-parallel over the 8 NeuronCores, 2048 rows each):
    wm = max(c, s, eq):  c = fg * (x >= thr), s = u[t_i, j] (outer
    product of 3 row-sel bits with 3 column masks, OR-combined),
    eq = onehot(label).
    sum(wm * A) with A = softplus(x) decomposes into
        sum over (t, j) of [ u * P1 + (1 - u) * P2 ]  +  per-row terms,
    where P1 = R^T A and P2 = (fg*R)^T (c' * A) are 8 x n_c matmul
    accumulations over all row tiles (R = onehot of the 8 sel-bit row
    types).  The per-row terms only need g_i = x[i, label_i], gathered
    with indirect DMA.  sum(wm * eq * x) = sum(g).
Per 256-row supertile the device does: 1 bf16 DMA load, ACT Exp +
ACT Ln(1+e) (softplus, batched into two table phases so the Exp/Ln
activation tables load twice per kernel), one DVE bf16 threshold
compare + one DVE bf16 multiply, and 12 PE matmuls accumulating into
PSUM.  Measured ~63 us on hardware per core (HBM roofline ~28 us for
f32; x is streamed bf16, ACT softplus at 2 passes/element is the
binding engine at ~40 us busy).
"""

import math
from contextlib import ExitStack

import numpy as np
import ml_dtypes

import concourse.bass as bass
import concourse.tile as tile
from concourse import bacc, mybir
from concourse.bass_utils import run_bass_kernel_spmd

N_I, N_C = 16384, 1231
N_CORES = 8
N_LOC = N_I // N_CORES          # 2048 rows per core
P = 128
K_TILES = N_LOC // P            # 16 row tiles per core
SUPER = 2                       # row tiles per supertile (ACT/DVE instr batching)
N_SUPER = K_TILES // SUPER
THR = float(math.log(0.7 / 0.3))  # sigmoid(x) >= 0.7  <=>  x >= THR
TAU = float(math.log(1.0 + 0.7 / 0.3))  # softplus(THR): x >= THR <=> softplus(x) >= TAU
N_CHUNKS = [(0, 512), (512, 1024), (1024, N_C)]  # PSUM-bank-sized matmul slices

F32 = mybir.dt.float32
BF16 = mybir.dt.bfloat16
I32 = mybir.dt.int32


def _build_nc():
    nc = bacc.Bacc(None, target_bir_lowering=False)
    # x streamed as bf16: halves HBM traffic; softplus/threshold precision
    # impact is ~1e-4 relative (unbiased rounding), far inside tolerance
    x = nc.dram_tensor("x", [N_LOC, N_C], BF16, kind="ExternalInput")
    r_d = nc.dram_tensor("r", [P, K_TILES, 8], BF16, kind="ExternalInput")
    rp_d = nc.dram_tensor("rp", [P, K_TILES, 8], BF16, kind="ExternalInput")
    rpt_d = nc.dram_tensor("rpt", [P, K_TILES, 8], BF16, kind="ExternalInput")
    u_d = nc.dram_tensor("u", [8, N_C], F32, kind="ExternalInput")
    uc_d = nc.dram_tensor("uc", [8, N_C], F32, kind="ExternalInput")
    a_d = nc.dram_tensor("wa", [P, K_TILES], F32, kind="ExternalInput")
    b_d = nc.dram_tensor("wb", [P, K_TILES], F32, kind="ExternalInput")
    goff_d = nc.dram_tensor("goff", [P, K_TILES], I32, kind="ExternalInput")
    out_d = nc.dram_tensor("out", [1, 1], F32, kind="ExternalOutput")

    xv = x.rearrange("(k p) c -> p k c", p=P)  # [128, K_TILES, N_C]
    x_flat = x.rearrange("r (c one) -> (r c) one", one=1)
    # supertile row-tile counts: small head tiles so ACT starts as soon as
    # the first 630KB lands instead of waiting for a full 1.26MB supertile;
    # small tail tiles so DVE/PE can chase the last Ln sooner
    SIZES = [SUPER] * 7 + [1, 1]
    assert sum(SIZES) == K_TILES
    STARTS = [sum(SIZES[:i]) for i in range(len(SIZES))]
    N_ST = len(SIZES)
    # one ACT table phase pair (all Exp then all Ln) -> 2 table loads;
    # bf16 x makes DMA fast enough that the E-phase is not starved
    PHASES = [list(range(0, N_ST))]

    with tile.TileContext(nc) as tc, ExitStack() as ctx:
        const = ctx.enter_context(tc.tile_pool(name="const", bufs=1))
        xpool = ctx.enter_context(tc.tile_pool(name="x", bufs=1))
        epool = ctx.enter_context(tc.tile_pool(name="e", bufs=1))
        apool = ctx.enter_context(tc.tile_pool(name="a", bufs=1))
        cpool = ctx.enter_context(tc.tile_pool(name="c", bufs=1))
        mpool = ctx.enter_context(tc.tile_pool(name="m", bufs=1))
        psum = ctx.enter_context(tc.tile_pool(name="psum", bufs=1, space="PSUM"))
        fin = ctx.enter_context(tc.tile_pool(name="fin", bufs=1))

        # first x supertile DMA goes out before any constant loads
        xs_tiles = [None] * N_ST

        def load_xs(s, split=False):
            k0, sz = STARTS[s], SIZES[s]
            xs_tiles[s] = xpool.tile([P, sz, N_C], BF16, tag="xs",
                                     name=f"xs{s}", bufs=4)
            if split and sz == 2:
                # halves on both DGE queues so the first tile lands sooner
                nc.sync.dma_start(xs_tiles[s][:, 0, :], xv[:, k0, :])
                nc.gpsimd.dma_start(xs_tiles[s][:, 1, :], xv[:, k0 + 1, :])
            else:
                nc.sync.dma_start(xs_tiles[s][:], xv[:, k0 : k0 + sz, :])

        load_xs(0, split=True)
        load_xs(1)

        # constants, issued from the otherwise-idle gpsimd queue so the
        # sync sequencer's ~0.6us/issue budget all goes to x tiles
        r_sb = const.tile([P, K_TILES, 8], BF16)
        nc.gpsimd.dma_start(r_sb[:], r_d[:])
        rp_sb = const.tile([P, K_TILES, 8], BF16)
        nc.gpsimd.dma_start(rp_sb[:], rp_d[:])
        rpt_sb = const.tile([P, K_TILES, 8], BF16)
        nc.gpsimd.dma_start(rpt_sb[:], rpt_d[:])
        goff_sb = const.tile([P, K_TILES], I32)
        nc.gpsimd.dma_start(goff_sb[:], goff_d[:])
        u_sb = const.tile([8, N_C], F32)
        nc.gpsimd.dma_start(u_sb[:], u_d[:])
        uc_sb = const.tile([8, N_C], F32)
        nc.gpsimd.dma_start(uc_sb[:], uc_d[:])
        a_sb = const.tile([P, K_TILES], F32)
        nc.gpsimd.dma_start(a_sb[:], a_d[:])
        b_sb = const.tile([P, K_TILES], F32)
        nc.gpsimd.dma_start(b_sb[:], b_d[:])
        ones = const.tile([P, 1], F32)
        nc.vector.memset(ones[:], 1.0)

        # per-row gathered logits g[p, k] = x[row, label[row]] — one
        # indirect DMA with all 2048 offsets (per-instruction overhead on
        # the gpsimd descriptor generator dominates split gathers)
        g_sb = const.tile([P, K_TILES], BF16)
        nc.gpsimd.indirect_dma_start(
            out=g_sb[:, :],
            out_offset=None,
            in_=x_flat,
            in_offset=bass.IndirectOffsetOnAxis(ap=goff_sb[:, :], axis=0),
        )

        p1 = psum.tile([8, N_C], F32, space="PSUM")
        p2 = psum.tile([8, N_C], F32, space="PSUM")

        eg = fin.tile([P, K_TILES], F32)
        spg = fin.tile([P, K_TILES], F32)

        # ACT instructions batched per phase (Exp xN then Ln xN) so the
        # activation-table swap happens 4x per kernel, not 20x.  The Tile
        # scheduler is table-load-oblivious, so the grouping is pinned
        # with explicit ordering deps between consecutive ACT instrs.
        act_order = []
        dve_order = []
        e_tiles = [None] * N_ST
        a_tiles = [None] * N_ST
        for pi, phase in enumerate(PHASES):
            for s in phase:
                if xs_tiles[s] is None:
                    load_xs(s)
                sz = SIZES[s]
                e_tiles[s] = epool.tile([P, sz, N_C], BF16, tag="e",
                                        name=f"et{s}", bufs=10)
                act_order.append(nc.scalar.activation(
                    e_tiles[s][:], xs_tiles[s][:], mybir.ActivationFunctionType.Exp
                ))
            last = pi == len(PHASES) - 1
            if last:
                # fold the tiny gathered-g softplus into the last phase's tables
                act_order.append(nc.scalar.activation(
                    eg[:], g_sb[:], mybir.ActivationFunctionType.Exp))
            for s in phase:
                sz = SIZES[s]
                a_tiles[s] = apool.tile([P, sz, N_C], BF16, tag="a",
                                        name=f"at{s}", bufs=4)
                act_order.append(nc.scalar.activation(
                    a_tiles[s][:], e_tiles[s][:],
                    mybir.ActivationFunctionType.Ln, bias=1.0,
                ))
            if last:
                act_order.append(nc.scalar.activation(
                    spg[:], eg[:], mybir.ActivationFunctionType.Ln, bias=1.0
                ))
            for s in phase:
                sz = SIZES[s]
                a_t = a_tiles[s]
                # c' * A == relu(A - tau) + tau*c'  (exact for 0/1 c'):
                # both factors come from 4x-mode tensor_scalars, and the
                # tau*c' matmul folds into the same PSUM accumulator with
                # host-scaled weights tau*R' -- no 2x tensor_tensor needed
                gm_t = mpool.tile([P, sz, N_C], BF16, tag="m",
                                  name=f"gmt{s}", bufs=3)
                dve_order.append(nc.vector.tensor_scalar(
                    gm_t[:], a_t[:], TAU, 0.0,
                    mybir.AluOpType.subtract, mybir.AluOpType.max))
                c_t = cpool.tile([P, sz, N_C], BF16, tag="c",
                                 name=f"ct{s}", bufs=3)
                dve_order.append(nc.vector.tensor_scalar(
                    c_t[:], a_t[:], TAU, None, mybir.AluOpType.is_ge
                ))
                # all P1 matmuls before P2's: the P2 chunks wait on DVE
                # output and would stall the PE stream ahead of ready work
                for j in range(sz):
                    k = STARTS[s] + j
                    for n0, n1 in N_CHUNKS:
                        nc.tensor.matmul(
                            p1[:, n0:n1], r_sb[:, k, :], a_t[:, j, n0:n1],
                            start=(k == 0), stop=(k == K_TILES - 1),
                        )
                for j in range(sz):
                    k = STARTS[s] + j
                    for n0, n1 in N_CHUNKS:
                        nc.tensor.matmul(
                            p2[:, n0:n1], rp_sb[:, k, :], gm_t[:, j, n0:n1],
                            start=(k == 0), stop=False,
                        )
                        nc.tensor.matmul(
                            p2[:, n0:n1], rpt_sb[:, k, :], c_t[:, j, n0:n1],
                            start=False, stop=(k == K_TILES - 1),
                        )

        # pin the ACT stream order so table-load batching survives scheduling
        for prev, nxt in zip(act_order, act_order[1:]):
            tile.add_dep_helper(nxt.ins, prev.ins, sync=False,
                                reason="ACT table-load grouping")

        # epilogue: sum(u * P1 + (1 - u) * P2); bf16 intermediates let the
        # add run in the DVE 2x mode and shave the serial tail
        t1 = fin.tile([8, N_C], BF16)
        dve_order.append(nc.vector.tensor_tensor(
            t1[:], p1[:], u_sb[:], mybir.AluOpType.mult))
        t2 = fin.tile([8, N_C], BF16)
        dve_order.append(nc.vector.tensor_tensor(
            t2[:], p2[:], uc_sb[:], mybir.AluOpType.mult))
        t3 = fin.tile([8, N_C], BF16)
        dve_order.append(nc.vector.tensor_tensor(
            t3[:], t1[:], t2[:], mybir.AluOpType.add))
        r8 = fin.tile([8, 1], F32)
        dve_order.append(nc.vector.reduce_sum(
            r8[:], t3[:], axis=mybir.AxisListType.X))

        # per-row terms: (wa + wb*[g<thr]) * softplus(g) - g
        g32 = fin.tile([P, K_TILES], F32)
        dve_order.append(nc.vector.tensor_copy(g32[:], g_sb[:]))
        mlt = fin.tile([P, K_TILES], F32)
        dve_order.append(nc.vector.tensor_scalar(
            mlt[:], g32[:], THR, None, mybir.AluOpType.is_lt))
        w1 = fin.tile([P, K_TILES], F32)
        dve_order.append(nc.vector.tensor_tensor(
            w1[:], mlt[:], b_sb[:], mybir.AluOpType.mult))
        w2 = fin.tile([P, K_TILES], F32)
        dve_order.append(nc.vector.tensor_tensor(
            w2[:], w1[:], a_sb[:], mybir.AluOpType.add))
        t4 = fin.tile([P, K_TILES], F32)
        dve_order.append(nc.vector.tensor_tensor(
            t4[:], w2[:], spg[:], mybir.AluOpType.mult))
        t5 = fin.tile([P, K_TILES], F32)
        dve_order.append(nc.vector.tensor_tensor(
            t5[:], t4[:], g32[:], mybir.AluOpType.subtract))
        rr = fin.tile([P, 1], F32)
        dve_order.append(nc.vector.reduce_sum(
            rr[:], t5[:], axis=mybir.AxisListType.X))

        # total = sum(r8) + sum(rr), via ones^T matmuls into one PSUM scalar
        s_ps = psum.tile([1, 1], F32, space="PSUM")
        nc.tensor.matmul(s_ps[:], ones[:8, :], r8[:], start=True, stop=False,
                         skip_group_check=True)
        nc.tensor.matmul(s_ps[:], ones[:], rr[:], start=False, stop=True,
                         skip_group_check=True)
        out_sb = fin.tile([1, 1], F32)
        nc.vector.tensor_copy(out_sb[:], s_ps[:])
        nc.sync.dma_start(out_d[:], out_sb[:])

    nc.finalize()
    return nc


_NC_CACHE = None


def _get_nc():
    global _NC_CACHE
    if _NC_CACHE is None:
        _NC_CACHE = _build_nc()
    return _NC_CACHE


def _prep_in_maps(cls_logits, labels, rare_mask, common_mask, freq_mask,
                  rare_sel, common_sel, freq_sel):
    x = np.ascontiguousarray(
        np.asarray(cls_logits, dtype=np.float32).astype(ml_dtypes.bfloat16))
    lab = np.asarray(labels).astype(np.int64)
    rm = np.asarray(rare_mask).astype(np.float32)
    cm = np.asarray(common_mask).astype(np.float32)
    fm = np.asarray(freq_mask).astype(np.float32)
    rs = np.asarray(rare_sel).astype(np.int64)
    cs = np.asarray(common_sel).astype(np.int64)
    fs = np.asarray(freq_sel).astype(np.int64)

    t = rs + 2 * cs + 4 * fs                      # row type in [0, 8)
    fg = (lab != 0).astype(np.float32)
    R = np.zeros((N_I, 8), np.float32)
    R[np.arange(N_I), t] = 1.0
    Rp = R * fg[:, None]

    u8 = np.zeros((8, N_C), np.float32)
    for tt in range(8):
        m = np.zeros(N_C, np.float32)
        if tt & 1:
            m = np.maximum(m, rm)
        if tt & 2:
            m = np.maximum(m, cm)
        if tt & 4:
            m = np.maximum(m, fm)
        u8[tt] = m

    h = u8[t, lab]                                # s value at the label column
    wa = (1.0 - h) * (1.0 - fg)
    wb = (1.0 - h) * fg

    loc = np.arange(N_LOC, dtype=np.int64)

    def fold(v):  # [N_LOC] -> [P, K_TILES] (partition-major)
        return np.ascontiguousarray(v.reshape(K_TILES, P).T)

    in_maps = []
    for c in range(N_CORES):
        rows = slice(c * N_LOC, (c + 1) * N_LOC)
        goff = loc * N_C + lab[rows]
        in_maps.append({
            "x": x[rows],
            "r": np.ascontiguousarray(
                R[rows].reshape(K_TILES, P, 8).transpose(1, 0, 2)
            ).astype(ml_dtypes.bfloat16),
            "rp": np.ascontiguousarray(
                Rp[rows].reshape(K_TILES, P, 8).transpose(1, 0, 2)
            ).astype(ml_dtypes.bfloat16),
            "rpt": np.ascontiguousarray(
                (TAU * Rp[rows]).reshape(K_TILES, P, 8).transpose(1, 0, 2)
            ).astype(ml_dtypes.bfloat16),
            "u": u8,
            "uc": np.ascontiguousarray(1.0 - u8),
            "wa": fold(wa[rows].astype(np.float32)),
            "wb": fold(wb[rows].astype(np.float32)),
            "goff": fold(goff).astype(np.int32),
        })
    return in_maps


def kernel(cls_logits, labels, rare_mask, common_mask, freq_mask,
           rare_sel, common_sel, freq_sel, _trace=False):
    in_maps = _prep_in_maps(cls_logits, labels, rare_mask, common_mask,
                            freq_mask, rare_sel, common_sel, freq_sel)
    nc = _get_nc()
    res = run_bass_kernel_spmd(nc, in_maps, core_ids=list(range(N_CORES)),
                               trace=_trace)
    total = np.float32(0.0)
    for c in range(N_CORES):
        total += res.results[c]["out"].reshape(())
    out = np.asarray(total / np.float32(N_I), dtype=np.float32)
    if _trace:
        return out, res
    return out
